# revision 1
# baseline (speedup 1.0000x reference)
"""Trainium2 Bass kernel for nn_GameTensor_27195732918735.

Computes out[i,j,b] = Hessian_z V_i(z_all[j,b]) for i != j, zeros on the
diagonal, where V_i(z) = W2[i] @ tanh(W1[i] @ z + b1[i]) + b2[i].

Analytic form used on-device:
    u = W1 z + b1;  th = tanh(u);  s_k = -2 W2_k th_k (1 - th_k^2)
    H = W1^T diag(s) W1  =  sum_k s_k w1_k w1_k^T

Per-core plan (8 cores, SPMD):
  core c owns agent i = c//2 and three (j, batch-half) "tasks" (the 12
  nonzero (i,j) cells x 2 batch halves = 24 half-cells / 8 cores = 3).
  On-chip: T[k, a*128+c] = W1[k,a] * W1[k,c] is precomputed once per core
  (agent-local), then each task's Hessians for its 128 batches are a single
  [k=256] x [b=128] x [(a,c)=16384] matmul H[b,(a,c)] = sum_k S[k,b] T[k,(a,c)]
  with perfectly contiguous output DMA. Diagonal zero blocks are written
  host-side (they are constants).
"""

import numpy as np

import concourse.bass as bass
import concourse.mybir as mybir
import concourse.tile as tile
from concourse import bacc
from concourse.bass_utils import run_bass_kernel_spmd

N, B, D = 4, 256, 128
H2 = 2 * D  # 256 hidden
NCORES = 8
NTASK = 3  # (j, half) tasks per core
HALF = B // 2  # 128 batches per task

# matmul operand dtype for the big S^T @ T matmuls:
#   "bf16"  : bfloat16 operands (1 cyc/row, ~0.3% rel err)
#   "fp16"  : float16 operands (1 cyc/row, ~5e-4 rel err, 2x DVE T-precompute)
#   "f32r"  : float32r operands (4-byte, 1 cyc/row at N>=512 per cost model)
#   "f32"   : plain float32 (4 cyc/row, exact)
MM_MODE = "f32r"

_F32 = mybir.dt.float32


def _mm_store_dtype():
    if MM_MODE == "bf16":
        return mybir.dt.bfloat16
    if MM_MODE == "fp16":
        return mybir.dt.float16
    if MM_MODE == "f32r":
        return mybir.dt.float32r
    return _F32


def _mm_view(ap):
    return ap


def _emit(tc, nc, w1c, w1t, b1c, w2s, zt, out):
    mmdt = _mm_store_dtype()
    Tanh = mybir.ActivationFunctionType.Tanh
    mult = mybir.AluOpType.mult
    add = mybir.AluOpType.add

    with (
        tc.tile_pool(name="consts", bufs=1) as consts,
        tc.tile_pool(name="tpool", bufs=1) as tpool,
        tc.tile_pool(name="small", bufs=4) as small,
        tc.tile_pool(name="stage", bufs=3) as stage_pool,
        tc.tile_pool(name="upsum", bufs=2, space="PSUM") as upsum,
        tc.tile_pool(name="psum", bufs=6, space="PSUM") as psum,
    ):
        # ---- load constants -------------------------------------------------
        w1c_sb = consts.tile([128, 2, 128], _F32)  # [k%128, kchunk, c]
        nc.sync.dma_start(w1c_sb, w1c)
        w1t_sb = consts.tile([128, 256], _F32)  # [d, k]
        nc.sync.dma_start(w1t_sb, w1t)
        b1_sb = consts.tile([128, 2], _F32)
        nc.sync.dma_start(b1_sb, b1c)
        w2s_sb = consts.tile([128, 2], _F32)  # -2*W2, [k%128, kchunk]
        nc.sync.dma_start(w2s_sb, w2s)
        zt_sb = consts.tile([128, NTASK, 128], _F32)  # [d, task, b]
        nc.sync.dma_start(zt_sb, zt.rearrange("t d b -> d t b"))

        if mmdt == mybir.dt.bfloat16:
            w1m = consts.tile([128, 2, 128], mmdt)
            nc.vector.tensor_copy(out=w1m, in_=w1c_sb)
        else:
            w1m = w1c_sb

        # ---- S[k, b] per task: s = -2*W2 * th * (1 - th^2) ------------------
        s_sb = consts.tile([128, NTASK, 2, 128], mmdt)  # [k%128, task, kchunk, b]
        for t in range(NTASK):
            for kc in range(2):
                ups = upsum.tile([128, 128], _F32)
                nc.tensor.matmul(
                    ups,
                    lhsT=w1t_sb[:, kc * 128 : (kc + 1) * 128],
                    rhs=zt_sb[:, t, :],
                    start=True,
                    stop=True,
                )
                th = small.tile([128, 128], _F32, tag="th")
                nc.scalar.activation(th, ups, Tanh, bias=b1_sb[:, kc : kc + 1])
                sq = small.tile([128, 128], _F32, tag="sq")
                nc.vector.tensor_tensor(sq, th, th, mult)
                nc.vector.tensor_scalar(sq, sq, -1.0, 1.0, mult, add)
                nc.vector.tensor_tensor(sq, th, sq, mult)
                nc.vector.tensor_scalar(
                    s_sb[:, t, kc, :], sq, w2s_sb[:, kc : kc + 1], None, mult
                )

        # ---- T[k, a*128+c] = W1[k,a] * W1[k,c], 8 a-values per DVE op -------
        AG = 8  # a-values per op
        TT = tpool.tile([128, 2, 16384], mmdt)
        for g in range(128 // AG):
            for kc in range(2):
                dst = TT[:, kc, g * AG * 128 : (g + 1) * AG * 128].rearrange(
                    "p (x y) -> p x y", x=AG
                )
                in0 = w1m[:, kc, None, :].to_broadcast((128, AG, 128))
                in1 = w1m[:, kc, g * AG : (g + 1) * AG, None].to_broadcast(
                    (128, AG, 128)
                )
                nc.vector.tensor_tensor(dst, in0, in1, mult)

        # ---- main: H[b, (a,c)] = sum_k S[k,b] T[k,(a,c)] --------------------
        out_flat = [out[t].rearrange("b a c -> b (a c)") for t in range(NTASK)]
        for t in range(NTASK):
            for g4 in range(8):  # 4 n-tiles of 512 -> one 1 MiB DMA
                stg = stage_pool.tile([128, 2048], _F32)
                for nn in range(4):
                    n = g4 * 4 + nn
                    ps = psum.tile([128, 512], _F32)
                    nc.tensor.matmul(
                        ps,
                        lhsT=_mm_view(s_sb[:, t, 0, :]),
                        rhs=_mm_view(TT[:, 0, n * 512 : (n + 1) * 512]),
                        start=True,
                        stop=False,
                    )
                    nc.tensor.matmul(
                        ps,
                        lhsT=_mm_view(s_sb[:, t, 1, :]),
                        rhs=_mm_view(TT[:, 1, n * 512 : (n + 1) * 512]),
                        start=False,
                        stop=True,
                    )
                    dst = stg[:, nn * 512 : (nn + 1) * 512]
                    if n % 3 == 2:
                        nc.scalar.copy(dst, ps)
                    else:
                        nc.vector.tensor_copy(out=dst, in_=ps)
                nc.sync.dma_start(out_flat[t][:, g4 * 2048 : (g4 + 1) * 2048], stg)


_NC_CACHE = {}


def _core_tasks(c):
    i = c // 2
    js = [j for j in range(N) if j != i]
    halves = [(j, h) for j in js for h in (0, 1)]
    return i, (halves[0:3] if c % 2 == 0 else halves[3:6])


def _build():
    key = MM_MODE
    if key in _NC_CACHE:
        return _NC_CACHE[key]
    nc = bacc.Bacc("TRN2", target_bir_lowering=False, debug=False, num_devices=NCORES)
    w1c = nc.dram_tensor("w1c", [128, 2, 128], _F32, kind="ExternalInput").ap()
    w1t = nc.dram_tensor("w1t", [128, 256], _F32, kind="ExternalInput").ap()
    b1c = nc.dram_tensor("b1c", [128, 2], _F32, kind="ExternalInput").ap()
    w2s = nc.dram_tensor("w2s", [128, 2], _F32, kind="ExternalInput").ap()
    zt = nc.dram_tensor("zt", [NTASK, 128, 128], _F32, kind="ExternalInput").ap()
    out = nc.dram_tensor("out", [NTASK, HALF, D, D], _F32, kind="ExternalOutput").ap()
    with tile.TileContext(nc) as tc:
        _emit(tc, nc, w1c, w1t, b1c, w2s, zt, out)
    nc.compile()
    _NC_CACHE[key] = nc
    return nc


# Options for test harness introspection (set by test.py, unused in grading).
_RUN_KWARGS = {}
_LAST_RESULT = None


def kernel(z_all, W1, b1, W2, b2):
    global _LAST_RESULT
    z_all = np.asarray(z_all, dtype=np.float32)
    W1 = np.asarray(W1, dtype=np.float32)
    b1 = np.asarray(b1, dtype=np.float32)
    W2 = np.asarray(W2, dtype=np.float32)

    nc = _build()

    in_maps = []
    metas = []
    for c in range(NCORES):
        i, tasks = _core_tasks(c)
        metas.append((i, tasks))
        w1i = W1[i]  # [256, 128]
        in_maps.append(
            {
                "w1c": np.ascontiguousarray(
                    w1i.reshape(2, 128, 128).transpose(1, 0, 2)
                ),
                "w1t": np.ascontiguousarray(w1i.T),
                "b1c": np.ascontiguousarray(b1[i].reshape(2, 128).T),
                "w2s": np.ascontiguousarray((-2.0 * W2[i, 0]).reshape(2, 128).T),
                "zt": np.ascontiguousarray(
                    np.stack(
                        [
                            z_all[j, h * HALF : (h + 1) * HALF, :].T
                            for (j, h) in tasks
                        ]
                    )
                ),
            }
        )

    res = run_bass_kernel_spmd(nc, in_maps, list(range(NCORES)), **_RUN_KWARGS)
    _LAST_RESULT = res

    full = np.zeros((N, N, B, D, D), dtype=np.float32)
    for c in range(NCORES):
        i, tasks = metas[c]
        o = res.results[c]["out"]  # [NTASK, HALF, D, D]
        for t, (j, h) in enumerate(tasks):
            full[i, j, h * HALF : (h + 1) * HALF] = o[t]
    return full



# revision 13
# speedup vs baseline: 2.0796x; 2.0796x over previous
"""Trainium2 Bass kernel for nn_GameTensor_27195732918735.

Computes out[i,j,b] = Hessian_z V_i(z_all[j,b]) for i != j, zeros on the
diagonal, where V_i(z) = W2[i] @ tanh(W1[i] @ z + b1[i]) + b2[i].

Analytic form used on-device:
    u = W1 z + b1;  th = tanh(u);  s_k = -2 W2_k th_k (1 - th_k^2)
    H = W1^T diag(s) W1  =  sum_k s_k w1_k w1_k^T

H is symmetric, so the device only computes one entry per unordered pair
(a, c).  Pairs are packed by circular diagonal: slot d in 0..64 holds
T[k, d, a] = W1[k, a] * W1[k, (a + d) % 128], built on DVE from a doubled
copy of W1 (plus a one-shifted copy for odd d, keeping every operand
stride-1 and 4B-aligned so the 2x_1P bf16 perf mode engages).  Per task the
Hessians for 128 batches are then H[b, col] = sum_k S[k, b] T2[k, col]
(bf16 matmuls, fp32 PSUM), staged to SBUF as bf16 and DMAd out.  The host
mirrors the packed pairs into the full [B, D, D] blocks with a gather LUT
and writes the diagonal zero blocks (both pure data movement).

Per-core plan (8 cores, SPMD): core c owns agent i = c//2 and three
(j, batch-half) tasks (12 nonzero (i,j) cells x 2 halves = 24 / 8 = 3).
"""

import numpy as np
import ml_dtypes

import concourse.bass as bass
import concourse.mybir as mybir
import concourse.tile as tile
from concourse import bacc
from concourse.bass_utils import run_bass_kernel_spmd

N, B, D = 4, 256, 128
H2 = 2 * D  # 256 hidden
NCORES = 8
NTASK = 3  # (j, half) tasks per core
HALF = B // 2  # 128 batches per task

# Packed-pair layout: 65 diagonal slots of 128 columns.
# Column order: [E0 O0 E1 O1 E2 O2 E3 O3 | TAIL] where E-chunk e holds even
# d = 16e..16e+14 (8 slots), O-chunk o holds odd d = 16o+1..16o+15 (8 slots),
# TAIL is the single d=64 slot. Total 8*1024 + 128 = 8320 columns.
NSLOT = 65
COLS = NSLOT * 128  # 8320
NCHUNK = 4  # E/O chunk pairs
GROUPS = 8  # main 1024-col psum groups (cols 0..8191)
TAILCOL = 8192

MM_MODE = "bf16"  # kept for test-harness compat; bf16 is the only mode

_F32 = mybir.dt.float32
_BF16 = mybir.dt.bfloat16

_AP = None  # bass_rust.AP class, resolved lazily


def _win_ap(tile_ap, base_off, nd, dstep):
    """Overlapping sliding-window AP: [128p][nd windows, stride dstep][128, 1].

    tile_ap must be a [128, R] view of an SBUF tile (partition stride = row
    length). Window w reads elements base_off + w*dstep + 0..127.
    """
    global _AP
    if _AP is None:
        _AP = type(tile_ap)
    pdim = [int(v) for v in list(tile_ap.ap)[0]]  # [partition_stride, 128]
    return _AP(
        tensor=tile_ap.tensor,
        offset=int(tile_ap.offset) + base_off,
        ap=[pdim, [dstep, nd], [1, 128]],
    )


def _emit(tc, nc, w1d, w1o, w1t, zt, b1c, w2s, out):
    Tanh = mybir.ActivationFunctionType.Tanh
    mult = mybir.AluOpType.mult
    add = mybir.AluOpType.add

    with (
        tc.tile_pool(name="consts", bufs=1) as consts,
        tc.tile_pool(name="tpool", bufs=1) as tpool,
        tc.tile_pool(name="small", bufs=4) as small,
        tc.tile_pool(name="warm", bufs=1) as warm,
        tc.tile_pool(name="stage", bufs=6) as stage_pool,
        tc.tile_pool(name="tstage", bufs=1) as tstage_pool,
        tc.tile_pool(name="upsum", bufs=2, space="PSUM") as upsum,
        tc.tile_pool(name="psum", bufs=3, space="PSUM") as psum,
    ):
        # ---- load constants -------------------------------------------------
        w1d_sb = consts.tile([128, 2, 256], _BF16)  # [k%128, kc, a doubled]
        nc.sync.dma_start(w1d_sb, w1d)
        w1o_sb = consts.tile([128, 2, 256], _BF16)  # doubled, shifted by one
        nc.sync.dma_start(w1o_sb, w1o)
        w1t_sb = consts.tile([128, 256], _BF16)  # [d, k]
        nc.sync.dma_start(w1t_sb, w1t)
        zt_sb = consts.tile([128, NTASK, 128], _BF16)  # [d, task, b]
        nc.sync.dma_start(zt_sb, zt.rearrange("t d b -> d t b"))
        b1_sb = consts.tile([128, 2], _F32)
        nc.sync.dma_start(b1_sb, b1c)
        w2s_sb = consts.tile([128, 2], _F32)  # -2*W2, [k%128, kc]
        nc.sync.dma_start(w2s_sb, w2s)

        # ---- PE warmup: ramp the p-state while DMAs land --------------------
        wz = warm.tile([128, 512], _BF16)
        nc.vector.memset(wz, 0)
        for _ in range(6):
            wps = psum.tile([128, 1024], _F32, tag="ps")
            nc.tensor.matmul(
                wps[:, 0:512], lhsT=wz[:, 0:128], rhs=wz, start=True, stop=True
            )

        # ---- TT tail slot (d=64), both kc: ready before tail groups ---------
        TT = tpool.tile([128, 2, COLS], _BF16)
        for kc in range(2):
            nc.vector.tensor_tensor(
                TT[:, kc, TAILCOL : TAILCOL + 128],
                w1d_sb[:, kc, 0:128],
                w1d_sb[:, kc, 64:192],
                mult,
            )

        # ---- S[k, b] per (task, kc): s = -2*W2 * th * (1 - th^2) ------------
        s_sb = consts.tile([128, NTASK, 2, 128], _BF16)
        for t in range(NTASK):
            for kc in range(2):
                ups = upsum.tile([128, 128], _F32, tag="ups")
                nc.tensor.matmul(
                    ups,
                    lhsT=w1t_sb[:, kc * 128 : (kc + 1) * 128],
                    rhs=zt_sb[:, t, :],
                    start=True,
                    stop=True,
                )
                th = small.tile([128, 128], _F32, tag="th")
                nc.scalar.activation(th, ups, Tanh, bias=b1_sb[:, kc : kc + 1])
                sq = small.tile([128, 128], _F32, tag="sq")
                nc.scalar.square(sq, th)
                nc.vector.tensor_scalar(sq, sq, -1.0, 1.0, mult, add)
                nc.vector.tensor_tensor(sq, th, sq, mult)
                nc.vector.tensor_scalar(
                    s_sb[:, t, kc, :], sq, w2s_sb[:, kc : kc + 1], None, mult
                )

        # ---- tail groups (cols 8192..8319) for all tasks: done early --------
        tstage = tstage_pool.tile([128, NTASK, 128], _BF16)
        for t in range(NTASK):
            ps = upsum.tile([128, 128], _F32, tag="ups")
            nc.tensor.matmul(
                ps, lhsT=s_sb[:, t, 0, :], rhs=TT[:, 0, TAILCOL:], start=True, stop=False
            )
            nc.tensor.matmul(
                ps, lhsT=s_sb[:, t, 1, :], rhs=TT[:, 1, TAILCOL:], start=False, stop=True
            )
            nc.scalar.copy(tstage[:, t, :], ps)
        nc.sync.dma_start(
            out[:, :, TAILCOL:].rearrange("t b c -> b t c"), tstage
        )

        # ---- TT chunks: E0 O0 E1 O1 ... (1024 cols each, per kc) ------------
        # E-chunk e: windows at offsets 16e + {0,2,..,14}; O-chunk o: same
        # offsets into the one-shifted copy. GPSIMD (idle otherwise; it cannot
        # read PSUM) takes the kc=1 half of the later chunks in parallel.
        for ch in range(NCHUNK):
            for par, src in ((0, w1d_sb), (1, w1o_sb)):
                col0 = ch * 2048 + par * 1024
                for kc in range(2):
                    dst = TT[:, kc, col0 : col0 + 1024].rearrange(
                        "p (w x) -> p w x", w=8
                    )
                    in0 = w1d_sb[:, kc, None, 0:128].to_broadcast((128, 8, 128))
                    in1 = _win_ap(src[:, kc, :], 16 * ch, 8, 2)
                    eng = nc.gpsimd if (ch >= 1 and kc == 1) else nc.vector
                    eng.tensor_tensor(dst, in0, in1, mult)

        # ---- main loop: 8 groups x 3 tasks, copies round-robined ------------
        # engine schedule for the 24 big PSUM->SBUF copies (GPSIMD cannot
        # read PSUM, so only Scalar and Vector serve these)
        cp_engines = [
            nc.scalar, nc.vector, nc.scalar,
            nc.scalar, nc.vector, nc.scalar,
            nc.scalar, nc.vector,
        ]
        cp_i = 0
        for gp in range(4):
            for t in range(NTASK):
                stg = stage_pool.tile([128, 2048], _BF16)
                for half in range(2):
                    g = gp * 2 + half
                    col0 = g * 1024
                    ps = psum.tile([128, 1024], _F32, tag="ps")
                    for nn in range(2):
                        sl = slice(nn * 512, (nn + 1) * 512)
                        c0 = col0 + nn * 512
                        nc.tensor.matmul(
                            ps[:, sl],
                            lhsT=s_sb[:, t, 0, :],
                            rhs=TT[:, 0, c0 : c0 + 512],
                            start=True,
                            stop=False,
                        )
                        nc.tensor.matmul(
                            ps[:, sl],
                            lhsT=s_sb[:, t, 1, :],
                            rhs=TT[:, 1, c0 : c0 + 512],
                            start=False,
                            stop=True,
                        )
                    eng = cp_engines[cp_i % len(cp_engines)]
                    cp_i += 1
                    if eng is nc.scalar:
                        nc.scalar.copy(stg[:, half * 1024 : (half + 1) * 1024], ps)
                    else:
                        eng.tensor_copy(
                            out=stg[:, half * 1024 : (half + 1) * 1024], in_=ps
                        )
                nc.sync.dma_start(out[t, :, gp * 2048 : (gp + 1) * 2048], stg)


_NC_CACHE = {}


def _core_tasks(c):
    i = c // 2
    js = [j for j in range(N) if j != i]
    halves = [(j, h) for j in js for h in (0, 1)]
    return i, (halves[0:3] if c % 2 == 0 else halves[3:6])


def _build():
    key = "v1"
    if key in _NC_CACHE:
        return _NC_CACHE[key]
    nc = bacc.Bacc("TRN2", target_bir_lowering=False, debug=False, num_devices=NCORES)
    w1d = nc.dram_tensor("w1d", [128, 2, 256], _BF16, kind="ExternalInput").ap()
    w1o = nc.dram_tensor("w1o", [128, 2, 256], _BF16, kind="ExternalInput").ap()
    w1t = nc.dram_tensor("w1t", [128, 256], _BF16, kind="ExternalInput").ap()
    zt = nc.dram_tensor("zt", [NTASK, 128, 128], _BF16, kind="ExternalInput").ap()
    b1c = nc.dram_tensor("b1c", [128, 2], _F32, kind="ExternalInput").ap()
    w2s = nc.dram_tensor("w2s", [128, 2], _F32, kind="ExternalInput").ap()
    out = nc.dram_tensor("out", [NTASK, HALF, COLS], _BF16, kind="ExternalOutput").ap()
    with tile.TileContext(nc) as tc:
        _emit(tc, nc, w1d, w1o, w1t, zt, b1c, w2s, out)
    nc.compile()
    _NC_CACHE[key] = nc
    return nc


def _slot_col(d):
    """Column of diagonal-slot d in the packed layout."""
    if d == 64:
        return TAILCOL
    if d % 2 == 0:
        de = d // 2
        return (de // 8) * 2048 + (de % 8) * 128
    do = (d - 1) // 2
    return (do // 8) * 2048 + 1024 + (do % 8) * 128


_LUT = None


def _lut():
    global _LUT
    if _LUT is None:
        a = np.arange(128)[:, None]
        c = np.arange(128)[None, :]
        g = (c - a) % 128
        d = np.where(g <= 64, g, 128 - g)
        base_a = np.where(g <= 64, np.broadcast_to(a, (128, 128)), c)
        slot = np.vectorize(_slot_col)(d)
        _LUT = (slot + base_a).astype(np.int32)
    return _LUT


# Options for test harness introspection (set by test.py, unused in grading).
_RUN_KWARGS = {}
_LAST_RESULT = None


def kernel(z_all, W1, b1, W2, b2):
    global _LAST_RESULT
    z_all = np.asarray(z_all, dtype=np.float32)
    W1 = np.asarray(W1, dtype=np.float32)
    b1 = np.asarray(b1, dtype=np.float32)
    W2 = np.asarray(W2, dtype=np.float32)

    nc = _build()
    bf = ml_dtypes.bfloat16

    in_maps = []
    metas = []
    for c in range(NCORES):
        i, tasks = _core_tasks(c)
        metas.append((i, tasks))
        w1i = W1[i].astype(bf)  # [256, 128]
        w1ck = w1i.reshape(2, 128, 128).transpose(1, 0, 2)  # [k%128, kc, a]
        w1dbl = np.concatenate([w1ck, w1ck], axis=2)  # [128, 2, 256]
        w1shf = np.concatenate(
            [w1ck[:, :, 1:], w1ck[:, :, :1], w1ck[:, :, 1:], w1ck[:, :, :1]], axis=2
        )  # w1o[p, kc, j] = w1[p, kc, (j+1)%128]
        in_maps.append(
            {
                "w1d": np.ascontiguousarray(w1dbl),
                "w1o": np.ascontiguousarray(w1shf),
                "w1t": np.ascontiguousarray(w1i.T),
                "zt": np.ascontiguousarray(
                    np.stack(
                        [
                            z_all[j, h * HALF : (h + 1) * HALF, :].T
                            for (j, h) in tasks
                        ]
                    ).astype(bf)
                ),
                "b1c": np.ascontiguousarray(b1[i].reshape(2, 128).T),
                "w2s": np.ascontiguousarray((-2.0 * W2[i, 0]).reshape(2, 128).T),
            }
        )

    res = run_bass_kernel_spmd(nc, in_maps, list(range(NCORES)), **_RUN_KWARGS)
    _LAST_RESULT = res

    lut = _lut()
    full = np.zeros((N, N, B, D, D), dtype=np.float32)
    for c in range(NCORES):
        i, tasks = metas[c]
        o = np.asarray(res.results[c]["out"]).astype(np.float32)  # [NTASK, HALF, COLS]
        for t, (j, h) in enumerate(tasks):
            full[i, j, h * HALF : (h + 1) * HALF] = o[t][:, lut]
    return full
